# revision 1
# baseline (speedup 1.0000x reference)
"""LoRA Linear layer on 8 Trainium2 NeuronCores.

Computes out = x @ W.T + bias + scaling * (x @ A.T) @ B.T for
x [4, 4096, 4096] f32, W [4096, 4096], bias [4096], A [16, 4096], B [4096, 16].

Strategy:
- Host: fold the rank-16 LoRA path into the weight (exact up to f32
  rounding): W_eff = W.T + scaling * (A.T @ B.T), layout [in, out].
- Shard data-parallel over the batch: 16384 rows of x split 8 x 2048.
  W_eff/bias replicated per core; no collectives.
- Per core: out_s[2048, 4096] = x_s @ W_eff + bias as an fp16 matmul with
  fp32 PSUM accumulation (scale-relative absmax error ~3e-4 vs f32).
- PE structure: x m-tile [128,128] is the stationary operand, reused for
  2 consecutive matmuls (2 n-tiles of 512) — the implicit LDWEIGHTS is
  skipped when the weights AP repeats (measured 136ns/MM same-stationary
  vs 222ns/MM with a fresh stationary per matmul).
- SBUF: W n-blocks of [4096, 1024] fp16 (32 chunks of [128,1024], 64KB/
  partition) double-buffered so the next block streams during the
  current block's compute; x streams per-m-tile as packed [128,32,128]
  chunks (host pre-packs for contiguous DMA). DMA streams use separate
  engines (W: sync/HWDGE, x: gpsimd, out: vector) so slot-wait
  head-of-line blocking can't cross streams.
"""

import numpy as np

IN_F = 4096
OUT_F = 4096
R = 16
SCALING = 32.0 / R
N_CORES = 8
M_TOTAL = 4 * 4096
M_CORE = M_TOTAL // N_CORES  # 2048

P = 128
KO = IN_F // P  # 32 contraction chunks
NW = 512  # matmul free dim (one PSUM bank of f32)
NJ = 2  # n-tiles per block (stationary reused NJ times)
NB = OUT_F // (NJ * NW)  # 4 n blocks
NBW = NJ * NW  # 1024 cols per block
MT = M_CORE // P  # 16 m tiles

_CACHE = {}


def _build_nc(repeats=1, fake_w=False):
    """repeats>1 replays the whole compute pass (W/x re-streamed) — used
    only for device-time measurement by test.py. fake_w=True reuses one
    W block everywhere (numerically wrong; isolates W-DMA stalls)."""
    import concourse.mybir as mybir
    import concourse.tile as tile
    from concourse import bacc

    nc = bacc.Bacc("TRN2", target_bir_lowering=False, debug=False,
                   num_devices=N_CORES)
    xk = nc.dram_tensor("xk", [MT, P, KO, P], mybir.dt.float16,
                        kind="ExternalInput").ap()
    w = nc.dram_tensor("w", [IN_F, OUT_F], mybir.dt.float16,
                       kind="ExternalInput").ap()
    biasr = nc.dram_tensor("biasr", [P, OUT_F], mybir.dt.float32,
                           kind="ExternalInput").ap()
    out = nc.dram_tensor("out", [M_CORE, OUT_F], mybir.dt.float32,
                         kind="ExternalOutput").ap()

    wr = w.rearrange("(ko p) n -> ko p n", p=P)

    # (rep, nb) blocks in execution order
    blocks = [(rep, nb) for rep in range(repeats) for nb in range(NB)]

    with tile.TileContext(nc) as tc:
        with (
            tc.tile_pool(name="xpool", bufs=3) as xpool,
            tc.tile_pool(name="wpool", bufs=66) as wpool,
            tc.tile_pool(name="bpool", bufs=1) as bpool,
            tc.tile_pool(name="opool", bufs=4) as opool,
            tc.tile_pool(name="pspool", bufs=8, space="PSUM") as pspool,
        ):
            bias_sb = bpool.tile([P, OUT_F], mybir.dt.float32, name="bias_sb")
            nc.gpsimd.dma_start(bias_sb[:], biasr)

            w_sb = {}

            def load_w_block(bi):
                rep, nb = blocks[bi]
                if fake_w and bi > 0:
                    w_sb[nb] = w_sb[blocks[0][1]]
                    return
                for ko in range(KO):
                    wt = wpool.tile([P, NBW], mybir.dt.float16,
                                    name=f"w{rep}_{nb}_{ko}", tag="w",
                                    bufs=66)
                    nc.sync.dma_start(
                        wt[:], wr[ko, :, nb * NBW:(nb + 1) * NBW])
                    w_sb.setdefault(nb, [None] * KO)
                    w_sb[nb] = (w_sb[nb] if len(w_sb[nb]) == KO else w_sb[nb])
                    w_sb[nb][ko] = wt

            # preload the first two blocks' W (double buffer warm)
            load_w_block(0)
            if len(blocks) > 1:
                load_w_block(1)

            for bi, (rep, nb) in enumerate(blocks):
                wts = w_sb[nb]
                for mt in range(MT):
                    xm = xpool.tile([P, KO, P], mybir.dt.float16,
                                    name=f"xm{rep}_{nb}_{mt}", tag="x")
                    nc.gpsimd.dma_start(xm[:], xk[mt])
                    psums = [
                        pspool.tile([P, NW], mybir.dt.float32,
                                    name=f"ps_{rep}_{nb}_{mt}_{nj}",
                                    tag="ps")
                        for nj in range(NJ)
                    ]
                    for ko in range(KO):
                        lhsT = xm[:, ko, :]
                        wt = wts[ko]
                        for nj in range(NJ):
                            nc.tensor.matmul(
                                psums[nj][:],
                                lhsT,
                                wt[:, nj * NW:(nj + 1) * NW],
                                start=(ko == 0),
                                stop=(ko == KO - 1),
                            )
                    m0 = mt * P
                    for nj in range(NJ):
                        c0 = nb * NBW + nj * NW
                        ot = opool.tile([P, NW], mybir.dt.float32,
                                        name=f"o_{rep}_{nb}_{mt}_{nj}",
                                        tag="o")
                        nc.vector.tensor_add(
                            ot[:], psums[nj][:], bias_sb[:, c0:c0 + NW])
                        nc.scalar.dma_start(
                            out[m0:m0 + P, c0:c0 + NW], ot[:])

                    # kick off block bi+2's W stream near the start of this
                    # block (its slots are free: block bi-1 fully consumed)
                    if mt == 1 and bi + 2 < len(blocks):
                        load_w_block(bi + 2)

    nc.compile()
    return nc


def _get_nc():
    if "nc" not in _CACHE:
        _CACHE["nc"] = _build_nc()
    return _CACHE["nc"]


def make_in_maps(x, weight, bias, lora_A, lora_B):
    """Host-side shard prep: returns the per-core input maps."""
    w_eff = weight.T.astype(np.float32) + np.float32(SCALING) * (
        lora_A.T.astype(np.float32) @ lora_B.T.astype(np.float32))
    w16 = w_eff.astype(np.float16)
    biasr = np.ascontiguousarray(
        np.broadcast_to(bias.astype(np.float32), (P, OUT_F)))
    xf = np.asarray(x, dtype=np.float32).reshape(M_TOTAL, IN_F)
    in_maps = []
    for c in range(N_CORES):
        xs = xf[c * M_CORE:(c + 1) * M_CORE]
        xT = np.ascontiguousarray(xs.T, dtype=np.float16)  # [IN_F, M_CORE]
        # pack to [m_tile, p, ko, m] so each m-tile is one contiguous DMA
        xk = np.ascontiguousarray(
            xT.reshape(KO, P, MT, P).transpose(2, 1, 0, 3))
        in_maps.append({"xk": xk, "w": w16, "biasr": biasr})
    return in_maps


def kernel(x, weight, bias, lora_A, lora_B):
    from concourse.bass_utils import run_bass_kernel_spmd

    nc = _get_nc()
    in_maps = make_in_maps(x, weight, bias, lora_A, lora_B)
    res = run_bass_kernel_spmd(nc, in_maps, core_ids=list(range(N_CORES)))
    _CACHE["last_result"] = res
    out = np.concatenate([r["out"] for r in res.results], axis=0)
    return out.reshape(4, 4096, OUT_F)



# revision 2
# speedup vs baseline: 5.2605x; 5.2605x over previous
"""LoRA Linear layer on 8 Trainium2 NeuronCores.

Computes out = x @ W.T + bias + scaling * (x @ A.T) @ B.T for
x [4, 4096, 4096] f32, W [4096, 4096], bias [4096], A [16, 4096],
B [4096, 16].

Strategy (v2 — W-stationary, x SBUF-resident):
- Host: fold the rank-16 LoRA path into the weight (exact up to f32
  rounding): W_eff = W.T + scaling * (A.T @ B.T), layout [in, out], fp16.
- Shard data-parallel over rows: 16384 rows of x split 8 x 2048 — no
  collectives. Each core computes out_core.T = W_eff.T @ x_core.T.
- Per core the PE runs 4096 matmuls [128k,128n]x[128k,512m]: stationary
  W tile [128,128] reused across MB=4 moving m-blocks (psum [128n,512m]
  per bank), accumulating over ko=32 contraction chunks. Measured
  sustained rate on this part is ~2.0 GHz (P0), so the matmul floor is
  ~265ns/MM including the psum-bank round-robin.
- LDWEIGHTS hiding: the next stationary's LDWEIGHTS is pre-issued two
  matmuls before its first use (explicit nc.tensor.ldweights), letting
  the load overlap the current stationary's stream. Verified
  numerically: the PE weight path double-buffers the load.
- x.T is fully SBUF-resident (32 chunks [128, 2048] fp16 = 128KB/part),
  loaded once over two DMA queues: total HBM traffic 80MB/core
  (x 16MB + W 32MB + out 32MB).
- W streamed as host-packed [128, 32, 128] column strips (contiguous
  8KB/partition DMA), triple-buffered, prefetched 2 blocks ahead.
- PSUM: one 4-bank tile [128, 2048] f32 per n-strip, double-buffered;
  drain = single DVE tensor_scalar_add (+bias along partitions) and a
  single contiguous 1MB out-DMA per strip.
- out is written transposed [OUT_F, M_CORE]; host transposes back.
"""

import numpy as np

IN_F = 4096
OUT_F = 4096
R = 16
SCALING = 32.0 / R
N_CORES = 8
M_TOTAL = 4 * 4096
M_CORE = M_TOTAL // N_CORES  # 2048

P = 128
KO = IN_F // P  # 32 contraction chunks
NT = OUT_F // P  # 32 n-tiles (stationary strips)
NW = 512  # matmul moving free dim (one PSUM bank of f32)
MB = M_CORE // NW  # 4 m-blocks per psum group

_CACHE = {}


def _build_nc(repeats=1, fake_w=False):
    """repeats>1 replays the whole compute pass (W re-streamed, x kept
    resident) — used only for device-time measurement. fake_w=True
    reuses strip 0 everywhere (numerically wrong; isolates W-DMA
    stalls)."""
    import concourse.mybir as mybir
    import concourse.tile as tile
    from concourse import bacc

    nc = bacc.Bacc("TRN2", target_bir_lowering=False, debug=False,
                   num_devices=N_CORES)
    xk = nc.dram_tensor("xk", [KO, P, M_CORE], mybir.dt.float16,
                        kind="ExternalInput").ap()
    # host-packed strips: wk[nt, p, ko, n] — contiguous 8KB/partition
    wk = nc.dram_tensor("wk", [NT, P, KO, P], mybir.dt.float16,
                        kind="ExternalInput").ap()
    biasn = nc.dram_tensor("biasn", [P, NT], mybir.dt.float32,
                           kind="ExternalInput").ap()
    out = nc.dram_tensor("out", [OUT_F, M_CORE], mybir.dt.float32,
                         kind="ExternalOutput").ap()

    blocks = [(rep, nt) for rep in range(repeats) for nt in range(NT)]

    with tile.TileContext(nc) as tc:
        with (
            tc.tile_pool(name="xpool", bufs=KO) as xpool,
            tc.tile_pool(name="wpool", bufs=3) as wpool,
            tc.tile_pool(name="bpool", bufs=1) as bpool,
            tc.tile_pool(name="opool", bufs=3) as opool,
            tc.tile_pool(name="pspool", bufs=2, space="PSUM") as pspool,
        ):
            bias_sb = bpool.tile([P, NT], mybir.dt.float32, name="bias_sb")
            nc.gpsimd.dma_start(bias_sb[:], biasn)

            # x resident: 32 chunks [128, M_CORE], alternating DMA queues
            xs = []
            for ko in range(KO):
                xt = xpool.tile([P, M_CORE], mybir.dt.float16,
                                name=f"x{ko}", tag="x", bufs=KO)
                eng = nc.gpsimd if ko % 2 == 0 else nc.scalar
                eng.dma_start(xt[:], xk[ko])
                xs.append(xt)

            w_sb = {}
            w_first = []

            def load_w_strip(bi):
                rep, nt = blocks[bi]
                if fake_w and bi > 0:
                    w_sb[bi] = w_first[0]
                    return
                wt = wpool.tile([P, KO, P], mybir.dt.float16,
                                name=f"w{rep}_{nt}", tag="w", bufs=3)
                nc.sync.dma_start(wt[:], wk[nt])
                w_sb[bi] = wt
                if bi == 0:
                    w_first.append(wt)

            load_w_strip(0)
            if len(blocks) > 1:
                load_w_strip(1)

            for bi, (rep, nt) in enumerate(blocks):
                wt = w_sb.pop(bi)
                ps_big = pspool.tile([P, M_CORE], mybir.dt.float32,
                                     name=f"ps_{rep}_{nt}", tag="ps",
                                     bufs=2)
                psums = [ps_big[:, mb * NW:(mb + 1) * NW]
                         for mb in range(MB)]
                # pre-issue the next stationary's LDWEIGHTS two MMs
                # before its first matmul so the load overlaps the
                # current stationary's stream (hides ~100-200ns/load).
                if bi == 0:
                    nc.tensor.ldweights(weights=wt[:, 0, :])
                for ko in range(KO):
                    lhsT = wt[:, ko, :]
                    for mb in range(MB):
                        nc.tensor.matmul(
                            psums[mb],
                            lhsT,
                            xs[ko][:, mb * NW:(mb + 1) * NW],
                            start=(ko == 0),
                            stop=(ko == KO - 1),
                        )
                        if mb == 1:
                            if ko + 1 < KO:
                                nc.tensor.ldweights(
                                    weights=wt[:, ko + 1, :])
                            elif bi + 1 < len(blocks):
                                nc.tensor.ldweights(
                                    weights=w_sb[bi + 1][:, 0, :])
                    # prefetch strip bi+2 early in this block's compute
                    if ko == 1 and bi + 2 < len(blocks):
                        load_w_strip(bi + 2)
                n0 = nt * P
                ot = opool.tile([P, M_CORE], mybir.dt.float32,
                                name=f"o_{rep}_{nt}", tag="o", bufs=3)
                nc.vector.tensor_scalar_add(
                    ot[:], ps_big[:], bias_sb[:, nt:nt + 1])
                nc.scalar.dma_start(out[n0:n0 + P, :], ot[:])

    nc.compile()
    return nc


def _get_nc():
    if "nc" not in _CACHE:
        _CACHE["nc"] = _build_nc()
    return _CACHE["nc"]


def make_in_maps(x, weight, bias, lora_A, lora_B):
    """Host-side shard prep: returns the per-core input maps."""
    w_eff = weight.T.astype(np.float32) + np.float32(SCALING) * (
        lora_A.T.astype(np.float32) @ lora_B.T.astype(np.float32))
    w16 = w_eff.astype(np.float16)
    # packed strips: wk[nt, p, ko, n] = w16[ko*P + p, nt*P + n]
    wkp = np.ascontiguousarray(
        w16.reshape(KO, P, NT, P).transpose(2, 1, 0, 3))
    biasn = np.ascontiguousarray(
        bias.astype(np.float32).reshape(NT, P).T)
    xf = np.asarray(x, dtype=np.float32).reshape(M_TOTAL, IN_F)
    in_maps = []
    for c in range(N_CORES):
        xs = xf[c * M_CORE:(c + 1) * M_CORE]
        xT = np.ascontiguousarray(xs.T, dtype=np.float16)  # [IN_F, M_CORE]
        xkc = xT.reshape(KO, P, M_CORE)
        in_maps.append({"xk": xkc, "wk": wkp, "biasn": biasn})
    return in_maps


def kernel(x, weight, bias, lora_A, lora_B):
    from concourse.bass_utils import run_bass_kernel_spmd

    nc = _get_nc()
    in_maps = make_in_maps(x, weight, bias, lora_A, lora_B)
    res = run_bass_kernel_spmd(nc, in_maps, core_ids=list(range(N_CORES)))
    _CACHE["last_result"] = res
    out = np.concatenate(
        [np.ascontiguousarray(r["out"].T) for r in res.results], axis=0)
    return out.reshape(4, 4096, OUT_F)
